# revision 12
# baseline (speedup 1.0000x reference)
"""Trainium2 Bass kernel for nn_Bert_Proj_CRF (BERT projection + CRF NLL).

v2 design (data-parallel, 8 cores x 8 sequences):
  - fp8 embedding table; transpose-gather with a PERMUTED token order so
    PSUM partition p holds 4 consecutive steps (s = 4p+gl) of each
    sequence -> the CRF chunk layout needs no DRAM shuffle at all.
  - Bias folded into PSUM via a ones-outer-product matmul; the scan runs
    on UNNORMALIZED exp(logits) (the per-token log-sum-exp terms cancel
    against the gold emission score), so no per-token softmax division
    and no per-token Ln.
  - Per-sequence pipeline: gather_b || {matmuls, exp, chunk-matrix build,
    2 in-lane tree levels, gold partials} of earlier sequences.
  - Cross-chunk combine: per-b SBUF fold (128 lanes -> 8 lanes of a
    64-lane tile), 4 tree levels, second fold to 8 lanes, 3 levels,
    then the final alpha/Z assembly. One rescale (after 32-step
    products) keeps f32 in range; its logs ride the folds.
"""

import os
import numpy as np
import ml_dtypes

import concourse.bass as bass
import concourse.bacc as bacc
import concourse.tile as tile
import concourse.mybir as mybir

V, D, T = 21128, 768, 4
B, S = 64, 512
NCORES = 8
BL = B // NCORES            # 8 sequences per core
NG = 32                     # (b, gl) pairs: bg = b*4 + gl
F32 = mybir.dt.float32
BF16 = mybir.dt.bfloat16
FP8 = mybir.dt.float8e4
I16 = mybir.dt.int16
AF = mybir.ActivationFunctionType
AL = mybir.AluOpType
AX = mybir.AxisListType

KV = os.environ.get('KV', '')
PLAIN8 = 'plain8' in KV     # flip if the fp8 transpose-gather is NOT 16-bit interleaved

# hostK column layout (f32, [128, HK])
HK_TRANS = 0     # 16: trans matrix (broadcast all partitions)
HK_START = 16    # 4
HK_END = 20      # 4
HK_C0M = 24      # 1: per-lane b (lanes 0..7): 1 - mask0_b
HK_MB = 25       # 32: mask*(s>=1) per (p, bg)
HK_OH4M = 57     # 128: onehot(target)*mask per (p, bg, t)
HK_GPRE = 185    # 32: host-gathered trans/start/end gold terms per (p, bg)
HK_INVI = 217    # 512: (1-mb)*(k==j) per (p, bg, k, j)
HK_BIAS = 729    # 128: row 0 only: bias per (b, gl, t) (same for all gl)
HK_ONESROW = 857 # 128: row 0 only: 1.0 (bias-matmul lhsT)
HK_ONESCOL = 985 # 1: all partitions: 1.0 (gold-matmul lhsT)
HK_SEL1 = 992    # 128: fold1 selection: [k, j*8+m] = (k == m*16+j)
HK_SEL2 = 1120   # 64: fold2 selection: [k, g*8+m] = (k == m*8+g), k<64
HK_EB = 1184     # 15: col 7 all-ones window strip (per-b gold row select)
HK = 1200


def fap(t, off, dims):
    """AP over tile t's partition dim with custom free dims (element units)."""
    base = t if isinstance(t, bass.AP) else t[:]
    return bass.AP(
        tensor=base.tensor,
        offset=base.offset + off,
        ap=[list(base.ap[0])] + [list(d) for d in dims],
    )


_CACHE = {}


def _build():
    if "nc" in _CACHE:
        return _CACHE["nc"]
    nc = bacc.Bacc()

    table_h = nc.dram_tensor("table", [V, D], FP8, kind="ExternalInput")
    gidx_h = nc.dram_tensor("gidx", [128, BL * 32], I16, kind="ExternalInput")
    hostk_h = nc.dram_tensor("hostk", [128, HK], F32, kind="ExternalInput")
    rhs8_h = nc.dram_tensor("rhs8", [128, 6 * NG], FP8, kind="ExternalInput")
    sel1_h = nc.dram_tensor("sel1", [128, 16 * 120], BF16, kind="ExternalInput")
    nll_h = nc.dram_tensor("nll", [BL], F32, kind="ExternalOutput")
    if 'debug' in KV:
        dbg_p1 = nc.dram_tensor("dbg_p1", [128, 128], BF16, kind="ExternalOutput")
        dbg_f2 = nc.dram_tensor("dbg_f2", [64, 256], F32, kind="ExternalOutput")
        dbg_g17 = nc.dram_tensor("dbg_g17", [64, 17], F32, kind="ExternalOutput")
        dbg_eu = nc.dram_tensor("dbg_eu", [128, 128], F32, kind="ExternalOutput")

    with tile.TileContext(nc) as tc:
        with (
            tc.tile_pool(name="consts", bufs=1) as cp,
            tc.tile_pool(name="xt", bufs=3) as xp,
            tc.tile_pool(name="work", bufs=1) as wp,
            tc.tile_pool(name="psum", bufs=1, space="PSUM") as pp,
            tc.tile_pool(name="psum2", bufs=1, space="PSUM") as pp2,
            tc.tile_pool(name="psum3", bufs=1, space="PSUM") as pp3,
            tc.tile_pool(name="psum4", bufs=1, space="PSUM") as pp4,
        ):
            # ---------------- t0 const loads (SP, in need-order) -------------
            gidx = cp.tile([128, BL * 32], I16)
            nc.sync.dma_start(out=gidx[:], in_=gidx_h[:])
            hostk = cp.tile([128, HK], F32)
            nc.sync.dma_start(out=hostk[:], in_=hostk_h[:])
            rhs8 = cp.tile([128, 6 * NG], FP8)
            nc.sync.dma_start(out=rhs8[:], in_=rhs8_h[:])
            sel1 = cp.tile([128, 16 * 120], BF16)
            nc.sync.dma_start(out=sel1[:], in_=sel1_h[:])

            # Act: exp of the small params (Exp table loads here, off-path)
            expT = wp.tile([128, 16], F32)
            nc.scalar.activation(out=expT[:], in_=fap(hostk, HK_TRANS, [[1, 16]]),
                                 func=AF.Exp)
            expS8 = wp.tile([BL, T], F32)
            nc.scalar.activation(out=expS8[:], in_=fap(hostk, HK_START, [[1, 4]])[0:BL],
                                 func=AF.Exp)
            expE8 = wp.tile([BL, T], F32)
            nc.scalar.activation(out=expE8[:], in_=fap(hostk, HK_END, [[1, 4]])[0:BL],
                                 func=AF.Exp)

            # DVE: mbexpT[p, bg, k, j] = mb[p,bg] * expT[k,j]
            mbexpT = wp.tile([128, NG * 16], F32)
            nc.vector.tensor_tensor(
                out=fap(mbexpT, 0, [[16, NG], [1, 16]]),
                in0=fap(hostk, HK_MB, [[1, NG], [0, 16]]),
                in1=fap(expT, 0, [[0, NG], [1, 16]]),
                op=AL.mult,
            )

            # PE: bias broadcast into PSUM: lg[p, bg*4+t] = bias[bg*4+t]
            lg_ps = pp.tile([128, NG * T], F32)
            nc.tensor.matmul(
                lg_ps[:],
                lhsT=fap(hostk, HK_ONESROW, [[1, 128]])[0:1],
                rhs=fap(hostk, HK_BIAS, [[1, 128]])[0:1],
                start=True, stop=False, skip_group_check=True,
            )

            # ---------------- per-sequence pipeline ----------------
            eu = wp.tile([128, NG * T], F32)       # exp(logits), scan emissions
            P1 = wp.tile([128, BL * 16], BF16)     # 4-step chunk products (bf16 for PE fold)
            F2_ps = pp3.tile([64, 16 * 16], F32)   # fold1 dest: lane (b,g), (j, ij)
            F3_ps = pp4.tile([BL, 8 * 17], F32)    # fold2 dest: lane b, (g, val)
            Mf = wp.tile([128, 64], F32)
            tmpA = wp.tile([128, 128], F32)
            A2 = wp.tile([128, 32], F32)
            tmp2 = wp.tile([128, 64], F32)
            em4 = wp.tile([128, 16], F32)
            emitk = wp.tile([128, 4], F32)
            ttb = wp.tile([128, 4], F32)
            gold_ps = pp2.tile([BL, 4], F32)

            def emit_fold1(bb):
                # 16 selection matmuls, accumulated over bb (PSUM base must be
                # 0/32/64): F2_ps[bb*8+g, j*16+ij] += P1[16g+j, bb*16+ij]
                for j in range(16):
                    nc.tensor.matmul(
                        fap(F2_ps, j * 16, [[1, 16]]),
                        lhsT=fap(sel1, j * 120 + 56 - 8 * bb, [[1, 64]]),
                        rhs=fap(P1, bb * 16, [[1, 16]]),
                        start=(bb == 0 and j == 0), stop=(bb == BL - 1),
                        skip_group_check=True,
                    )

            def emit_gold(bb):
                # row-select accumulate: gold_ps[bb, gl] += sum_p ttb[p, gl]
                nc.tensor.matmul(
                    gold_ps[:],
                    lhsT=fap(hostk, HK_EB + 7 - bb, [[1, 8]]),
                    rhs=ttb[:],
                    start=(bb == 0), stop=(bb == BL - 1), skip_group_check=True,
                )

            for b in range(BL):
                xt = xp.tile([128, 6, S], FP8, tag="xt")
                nc.gpsimd.dma_gather(
                    out_ap=xt[:],
                    in_ap=table_h[:],
                    idxs_ap=gidx[:, b * 32:(b + 1) * 32],
                    num_idxs=S,
                    num_idxs_reg=S,
                    elem_size=D,
                    transpose=True,
                )
                for gl in range(4):
                    og = fap(lg_ps, (b * 4 + gl) * 4, [[1, 4]])
                    if PLAIN8:
                        for c in range(6):
                            nc.tensor.matmul(
                                og,
                                lhsT=fap(xt, c * 512 + gl * 128, [[1, 128]]),
                                rhs=fap(rhs8, c * NG + b * 4, [[1, 4]]),
                                start=False, stop=(c == 5), skip_group_check=True,
                            )
                    else:
                        for c2 in range(3):
                            for e in range(2):
                                nc.tensor.matmul(
                                    og,
                                    lhsT=fap(xt, c2 * 1024 + gl * 256 + e, [[2, 128]]),
                                    rhs=fap(rhs8, (2 * c2 + e) * NG + b * 4, [[1, 4]]),
                                    start=False, stop=(c2 == 2 and e == 1),
                                    skip_group_check=True,
                                )
                if b > 0:
                    emit_fold1(b - 1)
                    emit_gold(b - 1)
                # Act: eu_b = exp(lp_b)  (unnormalized; includes bias)
                nc.scalar.activation(
                    out=fap(eu, b * 16, [[1, 16]]),
                    in_=fap(lg_ps, b * 16, [[1, 16]]),
                    func=AF.Exp,
                )
                # DVE: step matrices M[gl][k,j] = mbexpT * eu[gl, j] + invI
                nc.vector.tensor_tensor(
                    out=fap(Mf, 0, [[16, 4], [4, 4], [1, 4]]),
                    in0=fap(mbexpT, b * 64, [[16, 4], [4, 4], [1, 4]]),
                    in1=fap(eu, b * 16, [[4, 4], [0, 4], [1, 4]]),
                    op=AL.mult,
                )
                nc.vector.tensor_tensor(
                    out=fap(Mf, 0, [[16, 4], [1, 16]]),
                    in0=fap(Mf, 0, [[16, 4], [1, 16]]),
                    in1=fap(hostk, HK_INVI + b * 64, [[16, 4], [1, 16]]),
                    op=AL.add,
                )
                # L1: two pair products (M0*M1, M2*M3)
                nc.vector.tensor_tensor(
                    out=fap(tmpA, 0, [[16, 8], [4, 4], [1, 4]]),
                    in0=fap(Mf, 0, [[32, 2], [1, 16], [0, 4]]),
                    in1=fap(Mf, 16, [[32, 2], [0, 4], [1, 16]]),
                    op=AL.mult,
                )
                nc.vector.reduce_sum(
                    out=fap(A2, 0, [[4, 8], [1, 4]]),
                    in_=fap(tmpA, 0, [[16, 8], [1, 4], [4, 4]]),
                    axis=AX.X,
                )
                # L2: chunk product -> P1[:, b*16:(b+1)*16]
                nc.vector.tensor_tensor(
                    out=fap(tmp2, 0, [[16, 4], [4, 4], [1, 4]]),
                    in0=fap(A2, 0, [[32, 1], [1, 16], [0, 4]]),
                    in1=fap(A2, 16, [[32, 1], [0, 4], [1, 16]]),
                    op=AL.mult,
                )
                with nc.allow_low_precision(reason="bf16 chunk products for PE fold"):
                    nc.vector.reduce_sum(
                        out=fap(P1, b * 16, [[4, 4], [1, 4]]),
                        in_=fap(tmp2, 0, [[16, 4], [1, 4], [4, 4]]),
                        axis=AX.X,
                    )
                # gold partials: emit = sum_t lp*oh4m, + host-gathered terms
                nc.vector.tensor_tensor(
                    out=em4[:],
                    in0=fap(lg_ps, b * 16, [[1, 16]]),
                    in1=fap(hostk, HK_OH4M + b * 16, [[1, 16]]),
                    op=AL.mult,
                )
                nc.vector.reduce_sum(
                    out=emitk[:], in_=fap(em4, 0, [[4, 4], [1, 4]]), axis=AX.X,
                )
                nc.vector.tensor_tensor(
                    out=ttb[:], in0=emitk[:],
                    in1=fap(hostk, HK_GPRE + b * 4, [[1, 4]]),
                    op=AL.add,
                )

            # ---------------- endgame ----------------
            emit_fold1(BL - 1)
            emit_gold(BL - 1)
            # Act: prefetch the Ln table; pinned after exp_7 via data dep on eu_7
            lnscr = wp.tile([1, 1], F32)
            nc.scalar.activation(
                out=lnscr[:], in_=fap(eu, (BL - 1) * 16, [[1, 1]])[0:1], func=AF.Ln)

            # Pool: eu0 rows (Pool is free after gathers)
            eu0T = wp.tile([BL, 4], F32)
            nc.gpsimd.dma_start(
                out=eu0T[:], in_=fap(eu, 0, [[16, 8], [1, 4]])[0:1])

            # phase2 on F2 [64 lanes = (b,g), 16 matrices each]
            F2s = wp.tile([64, 256], F32)
            nc.vector.tensor_copy(out=F2s[:], in_=F2_ps[:])
            if 'debug' in KV:
                nc.sync.dma_start(out=dbg_p1[:], in_=P1[:])
                nc.sync.dma_start(out=dbg_f2[:], in_=F2s[:])
                nc.sync.dma_start(out=dbg_eu[:], in_=eu[:])
            t2 = wp.tile([64, 512], F32)
            G8 = wp.tile([64, 128], F32)
            nc.vector.tensor_tensor(
                out=fap(t2, 0, [[16, 32], [4, 4], [1, 4]]),
                in0=fap(F2s, 0, [[32, 8], [1, 16], [0, 4]]),
                in1=fap(F2s, 16, [[32, 8], [0, 4], [1, 16]]),
                op=AL.mult,
            )
            nc.vector.reduce_sum(
                out=fap(G8, 0, [[4, 32], [1, 4]]),
                in_=fap(t2, 0, [[16, 32], [1, 4], [4, 4]]),
                axis=AX.X,
            )
            G4 = wp.tile([64, 64], F32)
            nc.vector.tensor_tensor(
                out=fap(t2, 0, [[16, 16], [4, 4], [1, 4]]),
                in0=fap(G8, 0, [[32, 4], [1, 16], [0, 4]]),
                in1=fap(G8, 16, [[32, 4], [0, 4], [1, 16]]),
                op=AL.mult,
            )
            nc.vector.reduce_sum(
                out=fap(G4, 0, [[4, 16], [1, 4]]),
                in_=fap(t2, 0, [[16, 16], [1, 4], [4, 4]]),
                axis=AX.X,
            )
            # rescale the four 16-step products per lane (Act Ln domain is
            # +-2^64; 32-step products can exceed it); log the maxes
            rmax = wp.tile([64, 4], F32)
            nc.vector.reduce_max(
                out=rmax[:], in_=fap(G4, 0, [[16, 4], [1, 16]]), axis=AX.X)
            rrec = wp.tile([64, 4], F32)
            nc.vector.reciprocal(out=rrec[:], in_=rmax[:])
            nc.vector.tensor_tensor(
                out=fap(G4, 0, [[16, 4], [1, 16]]),
                in0=fap(G4, 0, [[16, 4], [1, 16]]),
                in1=fap(rrec, 0, [[1, 4], [0, 16]]),
                op=AL.mult,
            )
            lgs = wp.tile([64, 4], F32)
            nc.scalar.activation(out=lgs[:], in_=rmax[:], func=AF.Ln)
            G2 = wp.tile([64, 32], F32)
            nc.vector.tensor_tensor(
                out=fap(t2, 0, [[16, 8], [4, 4], [1, 4]]),
                in0=fap(G4, 0, [[32, 2], [1, 16], [0, 4]]),
                in1=fap(G4, 16, [[32, 2], [0, 4], [1, 16]]),
                op=AL.mult,
            )
            nc.vector.reduce_sum(
                out=fap(G2, 0, [[4, 8], [1, 4]]),
                in_=fap(t2, 0, [[16, 8], [1, 4], [4, 4]]),
                axis=AX.X,
            )
            # L4 -> G17[:, 0:16]; log sum -> G17[:, 16]
            G17 = wp.tile([64, 17], F32)
            nc.vector.tensor_tensor(
                out=fap(t2, 0, [[16, 4], [4, 4], [1, 4]]),
                in0=fap(G2, 0, [[1, 16], [0, 4]]),
                in1=fap(G2, 16, [[0, 4], [1, 16]]),
                op=AL.mult,
            )
            nc.vector.reduce_sum(
                out=fap(G17, 0, [[4, 4], [1, 4]]),
                in_=fap(t2, 0, [[16, 4], [1, 4], [4, 4]]),
                axis=AX.X,
            )
            nc.vector.reduce_sum(
                out=fap(G17, 16, [[1, 1]]), in_=lgs[:], axis=AX.X)

            if 'debug' in KV:
                nc.sync.dma_start(out=dbg_g17[:], in_=G17[:])
            # PE: fold2 -> F3_ps[b, g*17:] = G17[b*8+g, :]
            for g in range(8):
                nc.tensor.matmul(
                    fap(F3_ps, g * 17, [[1, 17]]),
                    lhsT=fap(hostk, HK_SEL2 + g * 8, [[1, 8]])[0:64],
                    rhs=G17[:],
                    start=(g == 0), stop=(g == 7), skip_group_check=True,
                )

            # phase3 on F3
            F3s = wp.tile([BL, 8 * 17], F32)
            nc.vector.tensor_copy(out=F3s[:], in_=F3_ps[:])
            t3 = wp.tile([BL, 256], F32)
            H4 = wp.tile([BL, 64], F32)
            nc.vector.tensor_tensor(
                out=fap(t3, 0, [[16, 16], [4, 4], [1, 4]]),
                in0=fap(F3s, 0, [[34, 4], [1, 16], [0, 4]]),
                in1=fap(F3s, 17, [[34, 4], [0, 4], [1, 16]]),
                op=AL.mult,
            )
            nc.vector.reduce_sum(
                out=fap(H4, 0, [[4, 16], [1, 4]]),
                in_=fap(t3, 0, [[16, 16], [1, 4], [4, 4]]),
                axis=AX.X,
            )
            H2 = wp.tile([BL, 32], F32)
            nc.vector.tensor_tensor(
                out=fap(t3, 0, [[16, 8], [4, 4], [1, 4]]),
                in0=fap(H4, 0, [[32, 2], [1, 16], [0, 4]]),
                in1=fap(H4, 16, [[32, 2], [0, 4], [1, 16]]),
                op=AL.mult,
            )
            nc.vector.reduce_sum(
                out=fap(H2, 0, [[4, 8], [1, 4]]),
                in_=fap(t3, 0, [[16, 8], [1, 4], [4, 4]]),
                axis=AX.X,
            )
            Ht = wp.tile([BL, 16], F32)
            nc.vector.tensor_tensor(
                out=fap(t3, 0, [[16, 4], [4, 4], [1, 4]]),
                in0=fap(H2, 0, [[1, 16], [0, 4]]),
                in1=fap(H2, 16, [[0, 4], [1, 16]]),
                op=AL.mult,
            )
            nc.vector.reduce_sum(
                out=fap(Ht, 0, [[4, 4], [1, 4]]),
                in_=fap(t3, 0, [[16, 4], [1, 4], [4, 4]]),
                axis=AX.X,
            )
            lgsum3 = wp.tile([BL, 1], F32)
            nc.vector.reduce_sum(
                out=lgsum3[:], in_=fap(F3s, 16, [[17, 8]]), axis=AX.X)

            # final: alpha0 = eu0*expS; Z = (alpha0 @ Htot) . expE
            a0 = wp.tile([BL, 4], F32)
            nc.vector.tensor_tensor(out=a0[:], in0=eu0T[:], in1=expS8[:], op=AL.mult)
            ta = wp.tile([BL, 16], F32)
            nc.vector.tensor_tensor(
                out=fap(ta, 0, [[4, 4], [1, 4]]),        # [j, k]
                in0=fap(a0, 0, [[0, 4], [1, 4]]),
                in1=fap(Ht, 0, [[1, 4], [4, 4]]),
                op=AL.mult,
            )
            av = wp.tile([BL, 4], F32)
            nc.vector.reduce_sum(
                out=av[:], in_=fap(ta, 0, [[4, 4], [1, 4]]), axis=AX.X)
            ze = wp.tile([BL, 4], F32)
            nc.vector.tensor_tensor(out=ze[:], in0=av[:], in1=expE8[:], op=AL.mult)
            Zt = wp.tile([BL, 1], F32)
            nc.vector.reduce_sum(out=Zt[:], in_=ze[:], axis=AX.X)
            # mask0 correction: (1-mask0) * ln(sum eu0)
            sm0 = wp.tile([BL, 1], F32)
            nc.vector.reduce_sum(out=sm0[:], in_=eu0T[:], axis=AX.X)
            lnsm0 = wp.tile([BL, 1], F32)
            nc.scalar.activation(out=lnsm0[:], in_=sm0[:], func=AF.Ln)
            c0c = wp.tile([BL, 1], F32)
            nc.vector.tensor_tensor(
                out=c0c[:], in0=lnsm0[:],
                in1=fap(hostk, HK_C0M, [[1, 1]])[0:BL], op=AL.mult)
            lnZ = wp.tile([BL, 1], F32)
            nc.scalar.activation(out=lnZ[:], in_=Zt[:], func=AF.Ln)
            norm = wp.tile([BL, 1], F32)
            nc.vector.tensor_tensor(out=norm[:], in0=lnZ[:], in1=lgsum3[:], op=AL.add)
            goldv = wp.tile([BL, 1], F32)
            nc.vector.reduce_sum(out=goldv[:], in_=gold_ps[:], axis=AX.X)
            nllp = wp.tile([BL, 1], F32)
            nc.vector.tensor_tensor(out=nllp[:], in0=norm[:], in1=goldv[:],
                                    op=AL.subtract)
            nc.vector.tensor_tensor(out=nllp[:], in0=nllp[:], in1=c0c[:],
                                    op=AL.subtract)
            nc.sync.dma_start(out=nll_h[:], in_=nllp[:])

    nc.compile()
    _CACHE["nc"] = nc
    return nc


def _prep_core(words, target, corpus, embed_f32, shared_W, shared_b,
               domain_A, domain_b, trans_m, start_scores, end_scores):
    w = np.asarray(words, np.int64)          # [BL, S]
    t = np.asarray(target, np.int64)

    # permuted gather order: position k <-> token s = 4*(k%128) + k//128
    kk = np.arange(S)
    perm = 4 * (kk % 128) + kk // 128        # s for each position k
    gidx = np.zeros((128, BL * 32), np.int16)
    for b in range(BL):
        il = w[b, perm].astype(np.int16)     # idxs[k]
        gidx[:16, b * 32:(b + 1) * 32] = il.reshape(32, 16).T

    # per-(p, bg) token tensors, s = 4p + gl
    # layout [p, b, gl]: a[b, 4p+gl] -> reshape(BL, 128, 4) transpose(1,0,2)
    def pm(a):
        return np.ascontiguousarray(
            np.asarray(a, np.float64).reshape(BL, 128, 4)
            .transpose(1, 0, 2).reshape(128, NG)).astype(np.float32)

    mask = (w != 0)
    sfirst = np.ones((BL, S)); sfirst[:, 0] = 0.0
    mb = pm(mask * sfirst)                   # [128, NG]

    # gold host-gathered terms
    trans = np.asarray(trans_m, np.float64)
    start = np.asarray(start_scores, np.float64)
    end = np.asarray(end_scores, np.float64)
    tr_vals = trans[t[:, :-1], t[:, 1:]] * mask[:, 1:]          # [BL, S-1]
    gpre_tok = np.zeros((BL, S))
    gpre_tok[:, 1:] += tr_vals
    gpre_tok[:, 0] += start[t[:, 0]]
    last_idx = np.maximum(mask.sum(1) - 1, 0)
    bidx = np.arange(BL)
    gpre_tok[bidx, last_idx] += end[t[bidx, last_idx]]
    gpre_tok[:, 0] -= 22.0
    gpre = pm(gpre_tok)

    # one-hot(target)*mask [p, bg, t]
    oh = (t[..., None] == np.arange(T)[None, None, :]) * mask[..., None]
    oh4m = np.ascontiguousarray(
        oh.reshape(BL, 128, 4, T).transpose(1, 0, 2, 3)
        .reshape(128, NG * T)).astype(np.float32)

    # invI [p, bg, k, j] = (1-mb)*(k==j)
    eye = np.eye(T).reshape(1, 1, T * T)
    invI = ((1.0 - mb)[:, :, None] * eye).reshape(128, NG * 16).astype(np.float32)

    # Z-scale shift: device expE = exp(end - ZSH) keeps Z inside the Act-Ln
    # domain (+-2^64); compensated exactly via gpre (gold side).
    ZSH = 22.0
    hostk = np.zeros((128, HK), np.float32)
    hostk[:, HK_TRANS:HK_TRANS + 16] = trans.reshape(-1)[None, :]
    hostk[:, HK_START:HK_START + 4] = start[None, :]
    hostk[:, HK_END:HK_END + 4] = end[None, :] - ZSH
    hostk[:BL, HK_C0M] = 1.0 - mask[:, 0]
    hostk[:, HK_MB:HK_MB + NG] = mb
    hostk[:, HK_OH4M:HK_OH4M + 128] = oh4m
    hostk[:, HK_GPRE:HK_GPRE + NG] = gpre
    hostk[:, HK_INVI:HK_INVI + 512] = invI
    bias = (np.asarray(shared_b, np.float64)[None, :]
            + np.asarray(domain_b, np.float64)[corpus])         # [BL, T]
    hostk[0, HK_BIAS:HK_BIAS + 128] = np.repeat(
        bias[:, None, :], 4, axis=1).reshape(-1)
    hostk[0, HK_ONESROW:HK_ONESROW + 128] = 1.0
    hostk[:, HK_ONESCOL] = 1.0
    sel1 = np.zeros((128, 16 * 120), ml_dtypes.bfloat16)
    for k in range(128):
        sel1[k, (k % 16) * 120 + 56 + k // 16] = 1.0
    for g in range(8):
        for m in range(8):
            hostk[m * 8 + g, HK_SEL2 + g * 8 + m] = 1.0
    hostk[:, HK_EB + 7] = 1.0

    # weights: w8[b, d, t] = domain_A[corpus_b] + shared_W, fp8
    w8 = (np.asarray(domain_A, np.float64)[corpus]
          + np.asarray(shared_W, np.float64)[None]).astype(np.float32)
    w8q = w8.astype(ml_dtypes.float8_e4m3)                      # [BL, D, T]
    rhs8 = np.zeros((128, 6 * NG), ml_dtypes.float8_e4m3)
    dd = np.arange(D)
    if PLAIN8:
        cc, pp_ = dd // 128, dd % 128        # d = c*128 + p
        for b in range(BL):
            rhs8[pp_[:, None], (cc * NG + b * 4)[:, None] + np.arange(T)] = w8q[b]
    else:
        u = dd // 2
        e = dd % 2
        cc, pp_ = u // 128, u % 128          # d = 2*(c2*128+p)+e
        ce = 2 * cc + e
        for b in range(BL):
            rhs8[pp_[:, None], (ce * NG + b * 4)[:, None] + np.arange(T)] = w8q[b]

    return gidx, hostk, rhs8, sel1


def kernel(_trace=False, **inputs):
    from concourse.bass_utils import run_bass_kernel_spmd

    words = np.asarray(inputs["words"])
    target = np.asarray(inputs["target"])
    corpus = np.asarray(inputs["corpus"])
    table8 = np.ascontiguousarray(
        np.asarray(inputs["embed_table"], np.float32).astype(ml_dtypes.float8_e4m3))

    nc = _build()
    in_maps = []
    for k in range(NCORES):
        sl = slice(k * BL, (k + 1) * BL)
        gidx, hostk, rhs8, sel1 = _prep_core(
            words[sl], target[sl], corpus[sl], inputs["embed_table"],
            inputs["shared_W"], inputs["shared_b"], inputs["domain_A"],
            inputs["domain_b"], inputs["trans_m"], inputs["start_scores"],
            inputs["end_scores"],
        )
        in_maps.append({
            "table": table8, "gidx": gidx, "hostk": hostk, "rhs8": rhs8,
            "sel1": sel1,
        })
    res = run_bass_kernel_spmd(
        nc, in_maps, core_ids=list(range(NCORES)), trace=_trace,
    )
    if _trace:
        print("exec_time_ns:", res.exec_time_ns,
              "mean:", res.mean_exec_time_ns,
              "trace:", (res.instructions_and_trace or (None, None))[1])
    out = np.concatenate([res.results[k]["nll"] for k in range(NCORES)])
    return out.astype(np.float32)
